# revision 1
# baseline (speedup 1.0000x reference)
"""Single-head causal attention (B=8, S=2048, D=1024, H=128) on 8 trn2 cores.

Data-parallel over batch (1 element per core). Per core:

P1 (projections, Q^T/K^T/V^T layouts):
  Q/K weights are pre-scaled by 32 (= sqrt(D)) on the host so the PSUM holds
  32*q directly.  Each of Q/K costs 1.5 bf16-equivalent PE passes:
    main:  xh(bf16) @ Wh(bf16)                     -- 1.0 cyc/row
    corr:  xl@Wh + xh@Wl in ONE fp8e4m3 DoubleRow  -- 0.5 cyc/row
  The fp8 limbs are pre-scaled into e4m3's range on the host with scales
  that cancel pairwise (xl*32 x Wh/32, xh/32 x Wl*32), so the correction
  accumulates straight into the main PSUM.  V is a single bf16 pass; its
  bias is folded into the ACT PSUM->SBUF copy (per-partition bias).  K's
  bias is dropped entirely (adds a per-query constant to scores =>
  softmax-invariant).  Q's bias is per-partition in the Q^T layout, so it
  rides the ACT hi-copy as a bias and enters the lo limb via a DVE
  scalar_tensor_tensor ((psum + bq) - hi) -- no bias matmul at all.

P2 (scores/softmax/PV), per 128-row strip:
  scores = (32q)^T_strip.T @ (32k)^T = 2^10 * s in PSUM via
    main:  qth(bf16) @ kth(bf16)                   -- 1.0 cyc/row
    corr:  ql8@kh8 + qh8@kl8 fp8 DoubleRow         -- 0.5 cyc/row
  where qh8/kh8 = e4m3(hi) (DVE SBUF->SBUF copy) and ql8/kl8 =
  e4m3(psum - hi) (DVE sub with fp8 output); all at consistent 2^10
  scale.  The causal
  mask is added on the diagonal tile via a transpose-mode accumulate.
  Row max on DVE, exp on ACT with scale = 32/1024 (absorbs the 2^10),
  accum_out produces row sums.  P stored bf16, P^T tiles via PE
  transpose + DVE copyback, PV accumulation with P^T stationary;
  host divides by row sums.  The final band's PV is split by columns:
  queries 1536:1920 accumulate early (no strip-15 dependency, overlapping
  strip 15's softmax), leaving only the 128-query tail group + one small
  DMA after the last P^T.

Scheduling: P1 chunk-pair 0 (all of Q/K/V) overlaps the DMA-bound ramp;
strips 0-3 and the pair-1 projections interleave as mutual gap-filler;
in P2 each band's PV is split into per-strip filler slots inside the
NEXT group so it hides under softmax latency (DVE max chain + ACT exp
chain are the per-strip pacers).  P^T copybacks run 3:1 DVE:ACT.  Row
sums ride the exp's accum_out; their sm->sums reduce is deferred one
strip to avoid head-of-line blocking the DVE queue.  Inputs are shipped
in SBUF-layout and loaded by ~16 large DMAs (each dma_start costs
~650ns serialized dispatch), the first few alternating between the SP
and ACT hardware DGE queues.

Numerics validated against the jax fp32 reference: rel_l2 ~ 4.3e-3
(gate 2e-2).  x^T is produced on the host during sharding.
"""
import os
import sys

sys.path.insert(0, "/opt/trn_rl_repo")
import numpy as np
import ml_dtypes

import concourse.bass as bass
import concourse.mybir as mybir
import concourse.tile as tile
from concourse import bacc
from concourse.bass_utils import run_bass_kernel_spmd
from concourse.masks import make_identity
from concourse.tile_rust import add_dep_helper

B, S, D, H = 8, 2048, 1024, 128
NK = D // 128          # 8 d-tiles
NS = S // 128          # 16 strips / t-tiles
CH = 512               # psum chunk width
NCH = S // CH          # 4 chunks across full seq
HD = S // 2

F32 = mybir.dt.float32
BF16 = mybir.dt.bfloat16
F8 = mybir.dt.float8e4
DR = mybir.MatmulPerfMode.DoubleRow

_NC_CACHE = {}


def _build():
    nc = bacc.Bacc()
    xh_d = nc.declare_dram_parameter("xh", [128, NK, S], BF16, isOutput=False)
    x8_d = nc.declare_dram_parameter("x8", [128, NK, 2, S], F8, isOutput=False)
    Wh_d = nc.declare_dram_parameter("Wh", [128, 3, NK, H], BF16, isOutput=False)
    W8_d = nc.declare_dram_parameter("W8", [128, 2, NK, 2, H], F8, isOutput=False)
    bq_d = nc.declare_dram_parameter("bq2", [128, 1], F32, isOutput=False)
    bv_d = nc.declare_dram_parameter("bv", [128, 1], F32, isOutput=False)
    out_d = nc.declare_dram_parameter("outT", [H, S], F32, isOutput=True)
    sums_d = nc.declare_dram_parameter("sums", [128, NS], F32, isOutput=True)

    MASK_DVE = os.environ.get("MASK_DVE", "0") == "1"
    CBSPLIT = os.environ.get("CBSPLIT", "0") == "1"

    with tile.TileContext(nc) as tc:
        with (
            tc.tile_pool(name="cons", bufs=1) as cons,
            tc.tile_pool(name="qkv", bufs=1) as qkv,
            tc.tile_pool(name="pp", bufs=2) as pp,
            tc.tile_pool(name="outp", bufs=4) as outp,
            tc.tile_pool(name="stat", bufs=int(os.environ.get("STB", "6"))) as stat,
        ):
            # ---- constants ----
            wh_all = cons.tile([128, 3, NK, H], BF16, tag="wh")
            w8_all = cons.tile([128, 2, NK, 2, H], F8, tag="w8")
            wh_sb = [wh_all[:, p] for p in range(3)]
            w8_sb = [w8_all[:, p] for p in range(2)]
            # k-tile 0 of Wq first: the very first matmul needs only this
            # 32KB slice, not the full weight load (issued on the ACT hwdge
            # queue inside the pool block when R2=1)
            if os.environ.get("R2", "0") != "1":
                nc.sync.dma_start(out=wh_all[:, 0, 0, :], in_=Wh_d[:, 0, 0, :])
            bq_sb = cons.tile([128, 1], F32, tag="bq")
            bv_sb = cons.tile([128, 1], F32, tag="bv")

            identb = cons.tile([128, 128], BF16, tag="identb")
            make_identity(nc, identb)
            identf = cons.tile([128, 128], mybir.dt.float32r if os.environ.get("MF32R", "0") == "1" else F32, tag="identf")
            make_identity(nc, identf)
            # maskT[t, s] = -1e30 where s < t; its PE transpose is the
            # additive causal mask for a diagonal score tile.
            MF32R = os.environ.get("MF32R", "0") == "1"
            maskT = cons.tile([128, 128], mybir.dt.float32r if MF32R else F32, tag="maskT")
            nc.gpsimd.memset(maskT, 0.0)
            nc.gpsimd.affine_select(
                out=maskT, in_=maskT, compare_op=mybir.AluOpType.is_ge,
                fill=-1e30, base=0, pattern=[[1, 128]], channel_multiplier=-1,
            )
            if MASK_DVE:
                mask_sb = cons.tile([128, 128], F32, tag="mask_sb")
                nc.gpsimd.memset(mask_sb, 0.0)
                nc.gpsimd.affine_select(
                    out=mask_sb, in_=mask_sb, compare_op=mybir.AluOpType.is_ge,
                    fill=-1e30, base=0, pattern=[[-1, 128]], channel_multiplier=1,
                )

            qth = qkv.tile([128, S], BF16, tag="qth")
            kth = qkv.tile([128, S], BF16, tag="kth")
            q8 = qkv.tile([128, 2, S], F8, tag="q8")   # [:,0]=ql8, [:,1]=qh8
            k8 = qkv.tile([128, 2, S], F8, tag="k8")   # [:,0]=kh8, [:,1]=kl8
            vt_bf = qkv.tile([128, S], BF16, tag="vt")
            v_sb = qkv.tile([128, NS, H], BF16, tag="v")
            sums_all = qkv.tile([128, NS], F32, tag="sums_all")

            with (
                tc.tile_pool(name="xtp", bufs=1) as xtp,
                tc.tile_pool(name="ps_a", bufs=int(os.environ.get("SCB", "6")), space="PSUM") as ps_a,
            ):
                # ---- P1: input DMA; ramp-ordered, few big copies (each
                # dma_start costs ~650ns of serialized dispatch) ----
                xh_all = xtp.tile([128, NK, S], BF16, tag="xha")
                x8_all = xtp.tile([128, NK, 2, S], F8, tag="x8a")
                xh = [xh_all[:, k] for k in range(NK)]
                x8 = [x8_all[:, k] for k in range(NK)]

                DDISP = os.environ.get("DDISP", "1") == "1"
                eng2 = nc.scalar if DDISP else nc.sync

                def ld_xh(eng, k0, k1, s0, s1):
                    eng.dma_start(out=xh_all[:, k0:k1, s0:s1],
                                  in_=xh_d[:, k0:k1, s0:s1])

                XQD = os.environ.get("XQD", "0") == "1"

                def ld_x8(eng, k0, k1, s0, s1):
                    if XQD:
                        # ship only the xl8 limb; xh8 = e4m3(xh/32) is a pure
                        # elementwise convert of already-loaded xh -> derive
                        # on the idle GpSimd engine, saving 2MB of DMA
                        eng.dma_start(out=x8_all[:, k0:k1, 0, s0:s1],
                                      in_=x8_d[:, k0:k1, 0, s0:s1])
                        for k in range(k0, k1):
                            nc.gpsimd.tensor_scalar_mul(
                                x8_all[:, k, 1, s0:s1], xh_all[:, k, s0:s1],
                                1.0 / 32.0)
                    else:
                        eng.dma_start(out=x8_all[:, k0:k1, :, s0:s1],
                                      in_=x8_d[:, k0:k1, :, s0:s1])

                # ramp loads alternate dispatch engines: each dma_start costs
                # ~650ns of serialized per-engine dispatch
                RV = os.environ.get("RV", "b")
                if os.environ.get("R2", "0") == "1":
                    eng2.dma_start(out=wh_all[:, 0, 0, :], in_=Wh_d[:, 0, 0, :])
                ld_xh(nc.sync, 0, 1, 0, CH)
                if RV == "f":
                    ld_xh(nc.sync, 0, 1, CH, HD)
                elif os.environ.get("WSPL", "0") == "1":
                    eng2.dma_start(out=wh_all[:, 1, 0, :], in_=Wh_d[:, 1, 0, :])
                    eng2.dma_start(out=wh_all[:, 2, 0, :], in_=Wh_d[:, 2, 0, :])
                    ld_xh(nc.sync, 0, 1, CH, HD)
                    eng2.dma_start(out=wh_all[:, 0, 1:NK, :], in_=Wh_d[:, 0, 1:NK, :])
                    eng2.dma_start(out=wh_all[:, 1, 1:NK, :], in_=Wh_d[:, 1, 1:NK, :])
                    eng2.dma_start(out=wh_all[:, 2, 1:NK, :], in_=Wh_d[:, 2, 1:NK, :])
                else:
                    eng2.dma_start(out=wh_all[:, 0, 1:NK, :], in_=Wh_d[:, 0, 1:NK, :])
                    ld_xh(nc.sync, 0, 1, CH, HD)
                    eng2.dma_start(out=wh_all[:, 1:3], in_=Wh_d[:, 1:3])
                if RV == "a":
                    ld_xh(nc.sync, 1, 2, 0, HD)
                    ld_xh(eng2, 2, 4, 0, HD)
                    ld_xh(nc.sync, 4, 8, 0, HD)
                    eng2.dma_start(out=w8_all, in_=W8_d[:, :])
                    ld_x8(nc.sync, 0, 4, 0, HD)
                    ld_x8(nc.sync, 4, 8, 0, HD)
                elif RV == "b":
                    ld_xh(nc.sync, 1, 2, 0, HD)
                    ld_xh(eng2, 2, 4, 0, HD)
                    ld_xh(nc.sync, 4, 6, 0, HD)
                    ld_xh(eng2, 6, 8, 0, HD)
                    eng2.dma_start(out=w8_all, in_=W8_d[:, :])
                    ld_x8(nc.sync, 0, 4, 0, HD)
                    ld_x8(nc.sync, 4, 8, 0, HD)
                elif RV == "f":
                    # tiny k0-only Wk/Wv slice first on the ACT queue so the
                    # serial DMA device isn't blocked ahead of the critical
                    # x chunks; bulk weight loads follow them
                    eng2.dma_start(out=wh_all[:, 1:3, 0, :], in_=Wh_d[:, 1:3, 0, :])
                    eng2.dma_start(out=wh_all[:, 0, 1:NK, :], in_=Wh_d[:, 0, 1:NK, :])
                    ld_xh(nc.sync, 1, 2, 0, HD)
                    eng2.dma_start(out=wh_all[:, 1:3, 1:NK, :], in_=Wh_d[:, 1:3, 1:NK, :])
                    ld_xh(nc.sync, 2, 4, 0, HD)
                    ld_xh(eng2, 4, 6, 0, HD)
                    ld_xh(nc.sync, 6, 8, 0, HD)
                    eng2.dma_start(out=w8_all, in_=W8_d[:, :])
                    ld_x8(nc.sync, 0, 4, 0, HD)
                    ld_x8(nc.sync, 4, 8, 0, HD)
                elif RV == "e":
                    # per-k pieces alternating hwdge queues
                    for k in range(1, NK):
                        ld_xh(nc.sync if k % 2 else eng2, k, k + 1, 0, HD)
                    eng2.dma_start(out=w8_all, in_=W8_d[:, :])
                    ld_x8(nc.sync, 0, 4, 0, HD)
                    ld_x8(eng2, 4, 8, 0, HD)
                elif RV == "c":
                    ld_xh(nc.sync, 1, 2, 0, HD)
                    ld_xh(eng2, 2, 3, 0, HD)
                    ld_xh(nc.sync, 3, 4, 0, HD)
                    ld_xh(eng2, 4, 6, 0, HD)
                    ld_xh(nc.sync, 6, 8, 0, HD)
                    eng2.dma_start(out=w8_all, in_=W8_d[:, :])
                    ld_x8(nc.sync, 0, 4, 0, HD)
                    ld_x8(eng2, 4, 8, 0, HD)
                else:
                    ld_xh(nc.sync, 1, 2, 0, HD)
                    ld_xh(eng2, 2, 4, 0, HD)
                    ld_xh(nc.sync, 4, 8, 0, HD)
                    eng2.dma_start(out=w8_all, in_=W8_d[:, :])
                    ld_x8(eng2, 0, 4, 0, HD)
                    ld_x8(nc.sync, 4, 8, 0, HD)
                # small constant loads deferred out of the hot ramp, on the
                # ACT hwdge queue so they don't delay the x second halves'
                # dispatch on the sync queue
                CQ = os.environ.get("CQ", "0") == "1"
                ceng = eng2 if CQ else nc.sync
                ceng.dma_start(out=bq_sb, in_=bq_d[:, :])
                ceng.dma_start(out=bv_sb, in_=bv_d[:, :])
                ld_xh(nc.sync, 0, 4, HD, S)
                ld_xh(nc.sync, 4, 8, HD, S)
                ld_x8(nc.sync, 0, 4, HD, S)
                ld_x8(nc.sync, 4, 8, HD, S)

                NWARM = int(os.environ.get("NWARM", "0"))
                if NWARM:
                    # dummy transposes while the first x DMA is in flight:
                    # keeps the PE clock-gate ramping so real matmuls start
                    # at a higher p-state
                    wps = ps_a.tile([128, 128], BF16, name="warm", tag="aux", bufs=2)
                    for _ in range(NWARM):
                        nc.tensor.matmul(wps, identb, identb,
                                         is_transpose=True, start=True, stop=True,
                                         skip_group_check=True)

                ptb = pp.tile([128, NS, S], BF16, tag="pt", bufs=1)
                strip_p = {}
                pending_sum = []
                PSB = int(os.environ.get("PSB", "3"))
                TRDMA = os.environ.get("TRDMA", "0") == "1"
                dma_chain = [None]

                def emit_proj_groups(groups):
                    psums = {}
                    for (c, p) in groups:
                        psums[(c, p)] = ps_a.tile([128, CH], F32, name=f"pj{c}_{p}", tag="ps")
                    if os.environ.get("PMAJ", "0") == "1" and len(groups) > 4:
                        for p in (0, 1, 2):
                            for k in range(NK):
                                for c in sorted(set(c_ for c_, p_ in groups if p_ == p)):
                                    rhs_h = xh[k][:, CH * c : CH * (c + 1)]
                                    nc.tensor.matmul(psums[(c, p)], wh_sb[p][:, k, :], rhs_h,
                                                     start=(k == 0),
                                                     stop=(p == 2 and k == NK - 1))
                    else:
                        for k in range(NK):
                            for (c, p) in groups:
                                rhs_h = xh[k][:, CH * c : CH * (c + 1)]
                                nc.tensor.matmul(psums[(c, p)], wh_sb[p][:, k, :], rhs_h,
                                                 start=(k == 0),
                                                 stop=(p == 2 and k == NK - 1))
                    for k in range(NK):
                        for (c, p) in groups:
                            if p < 2:
                                nc.tensor.matmul(psums[(c, p)], w8_sb[p][:, k, :, :],
                                                 x8[k][:, :, CH * c : CH * (c + 1)],
                                                 perf_mode=DR, start=False,
                                                 stop=(k == NK - 1))
                    PCV = os.environ.get("PCV", "0") == "1"
                    PCV8 = os.environ.get("PCV8", "2")
                    for (c, p) in groups:
                        sl = slice(CH * c, CH * (c + 1))
                        if p == 0:
                            # Q bias is per-partition in the Q^T layout: it
                            # rides the ACT hi-copy; the lo limb gets it via
                            # (psum + bq) - hi on DVE
                            nc.scalar.activation(qth[:, sl], psums[(c, p)],
                                                 mybir.ActivationFunctionType.Identity,
                                                 bias=bq_sb)
                            if PCV8 == "2":
                                nc.vector.tensor_copy(q8[:, 1, sl], qth[:, sl])
                            else:
                                nc.scalar.activation(q8[:, 1, sl], psums[(c, p)],
                                                     mybir.ActivationFunctionType.Copy)
                            nc.vector.scalar_tensor_tensor(
                                out=q8[:, 0, sl], in0=psums[(c, p)], scalar=bq_sb,
                                in1=qth[:, sl], op0=mybir.AluOpType.add,
                                op1=mybir.AluOpType.subtract)
                        elif p == 1:
                            if PCV:
                                nc.vector.tensor_copy(kth[:, sl], psums[(c, p)])
                            else:
                                nc.scalar.activation(kth[:, sl], psums[(c, p)],
                                                     mybir.ActivationFunctionType.Copy)
                            if PCV8 == "1":
                                nc.vector.tensor_copy(k8[:, 0, sl], psums[(c, p)])
                            elif PCV8 == "2":
                                nc.vector.tensor_copy(k8[:, 0, sl], kth[:, sl])
                            else:
                                nc.scalar.activation(k8[:, 0, sl], psums[(c, p)],
                                                     mybir.ActivationFunctionType.Copy)
                            nc.vector.tensor_sub(k8[:, 1, sl], psums[(c, p)], kth[:, sl])
                        else:
                            nc.scalar.activation(vt_bf[:, sl], psums[(c, p)],
                                                 mybir.ActivationFunctionType.Identity,
                                                 bias=bv_sb)

                def emit_vtransp(j4):
                    vstage = ps_a.tile([128, 512], BF16, name=f"vst{j4}", tag="aux", bufs=2)
                    for m in range(4):
                        j = j4 + m
                        nc.tensor.matmul(vstage[:, 128 * m : 128 * (m + 1)],
                                         vt_bf[:, 128 * j : 128 * (j + 1)], identb,
                                         is_transpose=True, start=True, stop=True,
                                         skip_group_check=True)
                    nc.scalar.activation(v_sb[:, j4 : j4 + 4, :], vstage,
                                         mybir.ActivationFunctionType.Copy)

                def emit_strip(i):
                    L = 128 * (i + 1)
                    qh_s = qth[:, 128 * i : 128 * (i + 1)]
                    q8_s = q8[:, :, 128 * i : 128 * (i + 1)]
                    spans = [(c0, min(c0 + CH, L)) for c0 in range(0, L, CH)]
                    nch = len(spans)
                    if os.environ.get("DG1", "0") == "1" and nch > 1:
                        spans = [spans[-1]] + spans[:-1]
                    scs = []
                    for c, (lo_, hi_) in enumerate(spans):
                        w = hi_ - lo_
                        sc = ps_a.tile([128, CH], F32, name=f"sc{i}_{lo_}", tag="ps")
                        nc.tensor.matmul(sc[:, :w], qh_s, kth[:, lo_:hi_],
                                         start=True, stop=False)
                        nc.tensor.matmul(sc[:, :w], q8_s, k8[:, :, lo_:hi_],
                                         perf_mode=DR, start=False,
                                         stop=(hi_ != L or MASK_DVE))
                        if hi_ == L:
                            if MASK_DVE:
                                nc.vector.tensor_tensor(
                                    out=sc[:, w - 128 : w], in0=sc[:, w - 128 : w],
                                    in1=mask_sb, op=mybir.AluOpType.add)
                            else:
                                scm = sc[:, w - 128 : w]
                                if MF32R:
                                    scm = scm.bitcast(mybir.dt.float32r)
                                nc.tensor.matmul(scm, maskT, identf,
                                                 is_transpose=True, start=False, stop=True,
                                                 skip_group_check=True)
                        scs.append((sc, lo_, w))
                    # row max over the strip
                    st = stat.tile([128, 8], F32, tag="st")
                    for c, (sc, lo_, w) in enumerate(scs):
                        nc.vector.reduce_max(out=st[:, c : c + 1], in_=sc[:, :w],
                                             axis=mybir.AxisListType.X)
                    mxs = stat.tile([128, 1], F32, tag="mxs")
                    nc.vector.reduce_max(out=mxs, in_=st[:, :nch], axis=mybir.AxisListType.X)
                    nbias = stat.tile([128, 1], F32, tag="nbias")
                    nc.vector.tensor_scalar_mul(nbias, mxs, -0.03125)
                    # exp (+ row sums) -> P bf16; psum holds 2^10*s, exp scale 32/1024
                    p_sb = pp.tile([128, S], BF16, tag="p", bufs=int(os.environ.get("PBUF", "5")))
                    strip_p[i] = p_sb
                    sm = stat.tile([128, 8], F32, tag="sm")
                    for c, (sc, lo_, w) in enumerate(sorted(scs, key=lambda t: t[1])):
                        nc.scalar.activation(
                            p_sb[:, lo_ : lo_ + w], sc[:, :w],
                            mybir.ActivationFunctionType.Exp,
                            bias=nbias, scale=0.03125, accum_out=sm[:, c : c + 1])
                    # the sm -> sums_all reduce is deferred one strip: emitted
                    # here it would head-of-line block the DVE queue behind
                    # this strip's exps, stalling the next strip's copybacks
                    if len(pending_sum) >= int(os.environ.get("SDEF", "1")):
                        flush_sums()
                    pending_sum.append((i, sm, nch))

                def flush_sums():
                    while pending_sum:
                        i0, sm0, nch0 = pending_sum.pop()
                        nc.vector.reduce_sum(out=sums_all[:, i0 : i0 + 1],
                                             in_=sm0[:, :nch0],
                                             axis=mybir.AxisListType.X)

                def emit_strip_pt(i):
                    p_sb = strip_p[i]
                    if TRDMA:
                        tr = nc.sync.dma_start(
                            out=ptb[:, 0 : i + 1, 128 * i : 128 * (i + 1)],
                            in_=p_sb[:, 0 : 128 * (i + 1)], transpose=True)
                        if dma_chain[0] is not None:
                            add_dep_helper(tr.ins, dma_chain[0], sync=True,
                                           reason="serialize xbar transposes")
                        dma_chain[0] = tr.ins
                        return
                    for j4 in range(0, i + 1, 4):
                        jn = min(4, i + 1 - j4)
                        tstage = ps_a.tile([128, 512], BF16, name=f"tst{i}_{j4}", tag="aux", bufs=2)
                        for m in range(jn):
                            j = j4 + m
                            nc.tensor.matmul(tstage[:, 128 * m : 128 * (m + 1)],
                                             p_sb[:, 128 * j : 128 * (j + 1)], identb,
                                             is_transpose=True, start=True, stop=True,
                                             skip_group_check=True)
                        dst = ptb[:, j4 : j4 + jn, 128 * i : 128 * (i + 1)]
                        srcv = tstage[:, : 128 * jn].rearrange("p (a b) -> p a b", b=128)
                        cbn = int(os.environ.get("CBN", "1"))
                        cbg = int(os.environ.get("CBG", "1"))
                        to_act = (cbn == 2 and (j4 // 4) % 2 == 1) or \
                                 (cbn == 1 and (j4 // 4) % 4 == cbg)
                        if i >= 16 - int(os.environ.get("CB15N", "1")) and \
                                os.environ.get("CB15", "1") == "1":
                            to_act = False
                        if to_act:
                            nc.scalar.activation(dst, srcv, mybir.ActivationFunctionType.Copy)
                        else:
                            nc.vector.tensor_copy(dst, srcv)

                band_oT = {}

                def emit_band_part(gi, js, split=False):
                    # partial PV accumulation for band gi over strip-tiles js;
                    # split=True runs 4 independent 128-col groups so the
                    # copies/DMAs pipeline with the PV tail (for the last band)
                    b_lo = 512 * gi
                    b_hi = 512 * (gi + 1)
                    njs = 4 * gi + 4
                    if gi not in band_oT:
                        if gi == 3 and os.environ.get("T3PS", "1") == "1":
                            band_oT[gi] = ps_a.tile([128, CH], F32, name=f"oT{gi}",
                                                    tag="ps",
                                                    bufs=int(os.environ.get("SCB", "6")))
                        else:
                            band_oT[gi] = ps_a.tile([128, CH], F32, name=f"oT{gi}",
                                                    tag=os.environ.get("OTT", "aux"),
                                                    bufs=int(os.environ.get("OTB", "2")))
                    oT = band_oT[gi]
                    if split:
                        # two independent accumulation groups by column half:
                        # half A's copy+DMA hide under half B's PV
                        osb = outp.tile([128, CH], F32, name=f"osb{gi}", tag="osb")
                        for pc in range(2):
                            plo, phi = b_lo + 256 * pc, b_lo + 256 * (pc + 1)
                            pjs = [j for j in js if 128 * j < phi]
                            for n_, j in enumerate(pjs):
                                lo = max(128 * j, plo) - plo
                                nc.tensor.matmul(oT[:, 256 * pc + lo : 256 * (pc + 1)],
                                                 v_sb[:, j, :], ptb[:, j, plo + lo : phi],
                                                 start=(n_ == 0), stop=(n_ == len(pjs) - 1),
                                                 skip_group_check=True)
                            if pc == 0:
                                nc.scalar.activation(osb[:, 0:256], oT[:, 0:256],
                                                     mybir.ActivationFunctionType.Copy)
                                nc.scalar.dma_start(out=out_d[:, plo:phi],
                                                    in_=osb[:, 0:256])
                            else:
                                nc.vector.tensor_copy(osb[:, 256:512], oT[:, 256:512])
                                nc.sync.dma_start(out=out_d[:, plo:phi],
                                                  in_=osb[:, 256:512])
                        return
                    for j in js:
                        lo = max(128 * j, b_lo) - b_lo
                        nc.tensor.matmul(oT[:, lo:], v_sb[:, j, :],
                                         ptb[:, j, b_lo + lo : b_hi],
                                         start=(j == 0), stop=(j == njs - 1),
                                         skip_group_check=True)
                    if js[-1] == njs - 1:
                        osb = outp.tile([128, CH], F32, name=f"osb{gi}", tag="osb")
                        if os.environ.get("OSBACT", "0") == "1":
                            nc.scalar.activation(osb, oT, mybir.ActivationFunctionType.Copy)
                            nc.sync.dma_start(out=out_d[:, b_lo:b_hi], in_=osb)
                        else:
                            nc.vector.tensor_copy(osb, oT)
                            nc.sync.dma_start(out=out_d[:, b_lo:b_hi], in_=osb)

                def emit_band(gi, split=False):
                    emit_band_part(gi, list(range(4 * gi + 4)), split=split)

                # chunk-pair 0 full (V included: it consumes the same early x
                # tiles, giving the PE more work per arriving tile during the
                # DMA-bound ramp), then early strips as gap-filler
                P1V = os.environ.get("P1V", "e")
                GORD = os.environ.get("GORD", "1") == "1"
                g0 = ([(0, 0), (0, 1), (0, 2), (1, 0), (1, 1), (1, 2)]
                      if GORD else [(c, p) for c in (0, 1) for p in range(3)])
                if P1V == "h":
                    # strips 4-7 need only K cols 0:1024 -> weave them into P1
                    # too; bands shift one group earlier
                    emit_proj_groups(g0)
                    emit_strip(0)
                    emit_strip(1)
                    emit_strip_pt(0)
                    emit_vtransp(0)
                    emit_proj_groups([(2, p) for p in (0, 1)])
                    emit_strip(2)
                    emit_strip_pt(1)
                    emit_proj_groups([(3, p) for p in (0, 1)])
                    emit_strip(3)
                    emit_strip_pt(2)
                    emit_vtransp(4)
                    emit_proj_groups([(c, 2) for c in (2, 3)])
                    emit_strip(4)
                    emit_strip_pt(3)
                    emit_strip(5)
                    emit_strip_pt(4)
                    emit_vtransp(8)
                    emit_strip(6)
                    emit_strip_pt(5)
                    emit_vtransp(12)
                    emit_strip(7)
                    emit_strip_pt(6)
                    fill2 = [[("b", 0, list(range(4)))],
                             [("b", 1, [0, 1, 2, 3])],
                             [("b", 1, [4, 5])],
                             [("b", 1, [6, 7])]]
                    fill3 = [[("b", 2, list(range(0, 3)))],
                             [("b", 2, list(range(3, 6)))],
                             [("b", 2, list(range(6, 9)))],
                             [("b", 2, list(range(9, 12)))]]
                    for gslots, r0 in ((fill2, 8), (fill3, 12)):
                        for idx, i in enumerate(range(r0, r0 + 4)):
                            emit_strip(i)
                            for (_, bg, js) in gslots[idx]:
                                emit_band_part(bg, js)
                            emit_strip_pt(i - 1)
                    emit_strip_pt(15)
                    flush_sums()
                    nc.sync.dma_start(out=sums_d[:, :], in_=sums_all)
                    emit_band(3)
                elif P1V == "a":
                    emit_proj_groups(g0)
                    for i in (0, 1, 2, 3):
                        emit_strip(i)
                    emit_vtransp(0)
                    emit_vtransp(4)
                    emit_proj_groups([(c, p) for c in (2, 3) for p in (0, 1)])
                    emit_proj_groups([(c, 2) for c in (2, 3)])
                    emit_vtransp(8)
                    emit_vtransp(12)
                elif P1V == "b":
                    emit_proj_groups(g0)
                    emit_strip(0)
                    emit_strip(1)
                    emit_vtransp(0)
                    emit_proj_groups([(c, p) for c in (2, 3) for p in (0, 1)])
                    emit_strip(2)
                    emit_strip(3)
                    emit_vtransp(4)
                    emit_proj_groups([(c, 2) for c in (2, 3)])
                    emit_vtransp(8)
                    emit_vtransp(12)
                elif P1V == "c":
                    emit_proj_groups(g0)
                    emit_vtransp(0)
                    emit_vtransp(4)
                    emit_proj_groups([(c, p) for c in (2, 3) for p in (0, 1)])
                    for i in (0, 1, 2, 3):
                        emit_strip(i)
                    emit_proj_groups([(c, 2) for c in (2, 3)])
                    emit_vtransp(8)
                    emit_vtransp(12)
                elif P1V == "d":  # split pair-1 q/k into two chunk-calls
                    emit_proj_groups(g0)
                    emit_strip(0)
                    emit_strip(1)
                    emit_vtransp(0)
                    emit_proj_groups([(2, p) for p in (0, 1)])
                    emit_strip(2)
                    emit_proj_groups([(3, p) for p in (0, 1)])
                    emit_strip(3)
                    emit_vtransp(4)
                    emit_proj_groups([(c, 2) for c in (2, 3)])
                    if os.environ.get("VTL", "0") != "1":
                        emit_vtransp(8)
                        emit_vtransp(12)
                elif P1V == "e":  # d + V split per chunk
                    emit_proj_groups(g0)
                    emit_strip(0)
                    emit_strip(1)
                    emit_vtransp(0)
                    emit_proj_groups([(2, p) for p in (0, 1)])
                    emit_strip(2)
                    emit_proj_groups([(3, p) for p in (0, 1)])
                    emit_strip(3)
                    emit_proj_groups([(2, 2)])
                    emit_vtransp(4)
                    emit_proj_groups([(3, 2)])
                    emit_vtransp(8)
                    emit_vtransp(12)
                elif P1V == "f":  # d but pair-0 also split in two calls
                    emit_proj_groups([(c, p) for c in (0, 1) for p in (0, 1)])
                    emit_proj_groups([(c, 2) for c in (0, 1)])
                    emit_strip(0)
                    emit_strip(1)
                    emit_vtransp(0)
                    emit_proj_groups([(2, p) for p in (0, 1)])
                    emit_strip(2)
                    emit_proj_groups([(3, p) for p in (0, 1)])
                    emit_strip(3)
                    emit_vtransp(4)
                    emit_proj_groups([(c, 2) for c in (2, 3)])
                    emit_vtransp(8)
                    emit_vtransp(12)
                else:  # g: d with vtransp moved later
                    emit_proj_groups(g0)
                    emit_strip(0)
                    emit_strip(1)
                    emit_proj_groups([(2, p) for p in (0, 1)])
                    emit_strip(2)
                    emit_proj_groups([(3, p) for p in (0, 1)])
                    emit_strip(3)
                    emit_proj_groups([(c, 2) for c in (2, 3)])
                    emit_vtransp(0)
                    emit_vtransp(4)
                    emit_vtransp(8)
                    emit_vtransp(12)
                # P^T emission deferred one strip so the next strip's maxes
                # outrank copybacks on DVE
                SCHED = os.environ.get("SCHED", "b")
                GORD = os.environ.get("GORD", "1") == "1"
                g0 = ([(0, 0), (0, 1), (0, 2), (1, 0), (1, 1), (1, 2)]
                      if GORD else [(c, p) for c in (0, 1) for p in range(3)])
                if P1V == "h":
                    pass
                elif SCHED == "a":
                    for i in (0, 1, 2, 3):
                        emit_strip_pt(i)
                    emit_band(0)
                    for g in (1, 2, 3):
                        prev = None
                        for i in range(4 * g, 4 * g + 4):
                            emit_strip(i)
                            if prev is not None:
                                emit_strip_pt(prev)
                            prev = i
                        emit_strip_pt(prev)
                        emit_band(g)
                else:
                    # band g-1's PV split into per-strip filler inside group g:
                    # it only depends on group g-1 copybacks, so it slots into
                    # group g's softmax-latency bubbles
                    for i in (0, 1, 2, 3):
                        emit_strip_pt(i)
                    TAIL3 = os.environ.get("TAIL3", "1") == "1"
                    BPAT = os.environ.get("BPAT", "b")
                    for g in (1, 2, 3):
                        njs = 4 * g  # tiles in band g-1
                        if BPAT == "f":      # front-heavy
                            cuts = [0, (njs + 1) // 2, njs, njs, njs]
                        elif BPAT == "b":    # back-heavy
                            cuts = [0, 0, njs // 3, 2 * njs // 3, njs]
                        elif BPAT == "c":    # all in last two slots
                            cuts = [0, 0, 0, njs // 2, njs]
                        elif BPAT == "d":    # skip slots 0-1, uneven tail
                            cuts = [0, 0, njs // 4, 5 * njs // 8, njs]
                        elif BPAT == "e":    # small slot-0 share
                            cuts = [0, njs // 8, njs // 2, 3 * njs // 4, njs]
                        elif BPAT == "g":    # slot-0 skip, even rest
                            cuts = [0, 0, njs // 4, njs // 2, njs]
                        elif BPAT == "h":    # b for g<3, even for g==3
                            if g == 3:
                                cuts = [njs * idx // 4 for idx in range(5)]
                            else:
                                cuts = [0, 0, njs // 3, 2 * njs // 3, njs]
                        elif BPAT == "i":    # b for g<3, front-heavy g==3
                            if g == 3:
                                cuts = [0, njs // 3, 2 * njs // 3, njs, njs]
                            else:
                                cuts = [0, 0, njs // 3, 2 * njs // 3, njs]
                        else:
                            cuts = [njs * idx // 4 for idx in range(5)]
                        parts = [list(range(cuts[idx], cuts[idx + 1]))
                                 for idx in range(4)]
                        prev = None
                        for idx, i in enumerate(range(4 * g, 4 * g + 4)):
                            emit_strip(i)
                            if g == 1 and idx in (2, 3) and os.environ.get("VTL", "0") == "1":
                                emit_vtransp(8 if idx == 2 else 12)
                            if parts[idx]:
                                emit_band_part(g - 1, parts[idx])
                            if prev is not None:
                                emit_strip_pt(prev)
                            prev = i
                        if g == 3 and TAIL3:
                            # split band 3 by COLUMNS: group A (cols
                            # 1536:1920) reads only strips <=14's P^T, so it
                            # runs while strip 15's softmax finishes; group B
                            # (cols 1920:2048, all 16 tiles) is the only PV
                            # left in the tail
                            oT = band_oT.setdefault(
                                3, ps_a.tile([128, CH], F32, name="oT3",
                                             tag="ps",
                                             bufs=int(os.environ.get("SCB", "6"))))
                            for n_, j in enumerate(range(15)):
                                lo = max(128 * j - 1536, 0)
                                nc.tensor.matmul(oT[:, lo:384], v_sb[:, j, :],
                                                 ptb[:, j, 1536 + lo : 1920],
                                                 start=(n_ == 0), stop=(n_ == 14),
                                                 skip_group_check=True)
                            emit_strip_pt(prev)
                            flush_sums()
                            nc.scalar.dma_start(out=sums_d[:, :], in_=sums_all)
                            osb = outp.tile([128, CH], F32, name="osb3", tag="osb")
                            nc.scalar.activation(osb[:, 0:384], oT[:, 0:384],
                                                 mybir.ActivationFunctionType.Copy)
                            nc.scalar.dma_start(out=out_d[:, 1536:1920],
                                                in_=osb[:, 0:384])
                            for n_, j in enumerate(range(16)):
                                nc.tensor.matmul(oT[:, 384:512], v_sb[:, j, :],
                                                 ptb[:, j, 1920:2048],
                                                 start=(n_ == 0), stop=(n_ == 15),
                                                 skip_group_check=True)
                            nc.vector.tensor_copy(osb[:, 384:512], oT[:, 384:512])
                            nc.sync.dma_start(out=out_d[:, 1920:2048],
                                              in_=osb[:, 384:512])
                        else:
                            emit_strip_pt(prev)
                    if not TAIL3:
                        flush_sums()
                        nc.sync.dma_start(out=sums_d[:, :], in_=sums_all)
                        emit_band(3, split=os.environ.get("BSPL", "0") == "1")
                if SCHED == "a":
                    flush_sums()
                    nc.sync.dma_start(out=sums_d[:, :], in_=sums_all)

    nc.compile()
    return nc


def _get_nc():
    key = (os.environ.get('SCB', '6'), os.environ.get('PBUF', '5'),
           os.environ.get('MASK_DVE', '0'), os.environ.get('CBN', '1'),
           os.environ.get('SCHED', 'b'), os.environ.get('OTT', 'aux'), os.environ.get('WQ0', '1'),
           os.environ.get('OTB', '2'))
    if key not in _NC_CACHE:
        _NC_CACHE[key] = _build()
    return _NC_CACHE[key]


def _bf16(a):
    return a.astype(ml_dtypes.bfloat16).astype(np.float32)


def _e4m3(a):
    return np.clip(a, -240, 240).astype(ml_dtypes.float8_e4m3)


def make_in_maps(x, Wq, bq, Wk, bk, Wv, bv):
    x = np.asarray(x, np.float32)
    xt = np.ascontiguousarray(x.transpose(0, 2, 1))  # [B, D, S]
    SC = np.float32(32.0)

    xh = _bf16(xt)
    xl = xt - xh
    x8 = np.stack([_e4m3(xl * SC), _e4m3(xh / SC)], axis=2)  # [B, D, 2, S]
    x8 = x8.reshape(B, NK, 128, 2, S).transpose(0, 2, 1, 3, 4)  # [B,128,NK,2,S]
    xh_bf = xh.astype(ml_dtypes.bfloat16).reshape(B, NK, 128, S).transpose(0, 2, 1, 3)

    m_all = {}
    whs, w8s = [], []
    for p, (W, n) in enumerate(((Wq, "q"), (Wk, "k"), (Wv, "v"))):
        W = np.asarray(W, np.float32)
        if n != "v":
            W = W * SC
        Whf = _bf16(W)
        whs.append(Whf.astype(ml_dtypes.bfloat16).reshape(NK, 128, H).transpose(1, 0, 2))
        if n != "v":
            Wl = W - Whf
            w8 = np.stack([_e4m3(Whf / SC), _e4m3(Wl * SC)], axis=1)  # [D, 2, H]
            w8s.append(w8.reshape(NK, 128, 2, H).transpose(1, 0, 2, 3))
    m_all["Wh"] = np.ascontiguousarray(np.stack(whs, axis=1))       # [128,3,NK,H]
    m_all["W8"] = np.ascontiguousarray(np.stack(w8s, axis=1))       # [128,2,NK,2,H]
    m_all["bq2"] = (np.asarray(bq, np.float32) * SC).reshape(128, 1)
    m_all["bv"] = np.asarray(bv, np.float32).reshape(128, 1)

    in_maps = []
    for bi in range(B):
        m = dict(m_all)
        m["xh"] = np.ascontiguousarray(xh_bf[bi])
        m["x8"] = np.ascontiguousarray(x8[bi])            # [D, 2, S]
        in_maps.append(m)
    return in_maps


def kernel(x, Wq, bq, Wk, bk, Wv, bv):
    nc = _get_nc()
    in_maps = make_in_maps(x, Wq, bq, Wk, bk, Wv, bv)
    res = run_bass_kernel_spmd(nc, in_maps, list(range(B)))
    outs = []
    for b in range(B):
        oT = res.results[b]["outT"]            # [H, S]
        sums = res.results[b]["sums"]          # [128, NS], s = 128*i + p
        s_flat = sums.T.reshape(S)
        outs.append((oT / s_flat[None, :]).T)
    return np.stack(outs).astype(np.float32)

